# revision 34
# baseline (speedup 1.0000x reference)
"""Multi-head attention (whisper-style, returns (out, qk)) on 8 Trainium2 cores.

Sharding: core c -> (batch b = c//2, head-group hg = c%2). Each core computes
8 heads (512 features) of one batch: QKV projections, causal attention scores
(returned as qk), softmax, attention-weighted V, and a partial output
projection. Host sums the two head-group partials per batch and adds bo.

All matmuls run in float32r (TF32-like fast path). Heads are processed in
pairs: the two heads of a qT/kT tile live at partitions 0-63 / 64-127, and
their K=64 score matmuls are packed into the PE array concurrently via
tile_position row groups, writing adjacent PSUM banks. exp / copies / causal
masking / DMA are fused across the pair with 3D access patterns.
"""

import sys

sys.path.insert(0, "/opt/trn_rl_repo")

import numpy as np

import concourse.bass as bass  # noqa: F401  (import registers AP machinery)
from concourse import bacc, bass_utils, mybir
import concourse.tile as tile

B, T, D, H = 4, 1500, 1024, 16
DH = D // H              # 64
NCORES = 8
HPC = H // 2             # 8 heads per core
FPC = HPC * DH           # 512 features per core
NT = (T + 127) // 128    # 12 partition tiles over T (last has 92 rows)
# Column chunk boundaries over T, aligned to the 512-float fp32 PSUM bank
# (a matmul output must not cross a bank boundary).
CS = [0, 512, 1024, T]
NJ = len(CS) - 1
VW = DH + 1              # v columns per head incl. ones column (65)
NV = HPC * VW            # 520

f32 = mybir.dt.float32
f32r = mybir.dt.float32r
EXP = mybir.ActivationFunctionType.Exp
LN = mybir.ActivationFunctionType.Ln
IDENT = mybir.ActivationFunctionType.Identity
GE = mybir.AluOpType.is_ge

_cached_nc = None


def _trows(i):
    return min(128, T - 128 * i)


def _build():
    nc = bacc.Bacc("TRN2", target_bir_lowering=False, debug=False)

    xT = nc.dram_tensor("xT", [D, T], f32r, kind="ExternalInput").ap()
    wqT = nc.dram_tensor("wqT", [D, FPC], f32r, kind="ExternalInput").ap()
    wkT = nc.dram_tensor("wkT", [D, FPC], f32r, kind="ExternalInput").ap()
    wvT = nc.dram_tensor("wvT", [D, FPC], f32r, kind="ExternalInput").ap()
    woT = nc.dram_tensor("woT", [FPC, D], f32r, kind="ExternalInput").ap()
    bqv = nc.dram_tensor("bq", [FPC, 1], f32, kind="ExternalInput").ap()
    bvv = nc.dram_tensor("bv", [1, FPC], f32, kind="ExternalInput").ap()
    vones = nc.dram_tensor("vones", [128, HPC, 1], f32r, kind="ExternalInput").ap()
    qk_out = nc.dram_tensor("qk_out", [HPC, T, T], f32, kind="ExternalOutput").ap()
    outT = nc.dram_tensor("outT", [D, T], f32, kind="ExternalOutput").ap()

    with tile.TileContext(nc) as tc:
        # ---------------- persistent SBUF ----------------
        with tc.tile_pool(name="perm", bufs=1) as perm:
            qT_sb = [perm.tile([128, T], f32r, name=f"qT{m}") for m in range(4)]
            kT_sb = [perm.tile([128, T], f32r, name=f"kT{m}") for m in range(4)]
            v_sb = [perm.tile([128, NV], f32r, name=f"v{i}") for i in range(NT)]
            oT_sb = [perm.tile([128, T], f32r, name=f"oT{m}") for m in range(4)]

            # ---------------- phase 1: QKV projections ----------------
            with tc.tile_pool(name="xw", bufs=1) as xw:
                xT_sb = [xw.tile([128, T], f32r, name=f"x{k}") for k in range(8)]
                for k in range(8):
                    nc.sync.dma_start(xT_sb[k][:], xT[128 * k : 128 * (k + 1), :])

                # q^T = (Wq_p*s) @ x^T (+ bq*s via ACT bias on evacuation)
                # k^T = (Wk_p*s) @ x^T
                with tc.tile_pool(name="wq", bufs=1) as wq, \
                     tc.tile_pool(name="qkps", bufs=2, space="PSUM") as qkps:
                    wq_sb = [wq.tile([128, FPC], f32r, name=f"wq{k}") for k in range(8)]
                    bq_sb = [wq.tile([128, 1], f32, name=f"bq{m}") for m in range(4)]
                    for k in range(8):
                        nc.sync.dma_start(wq_sb[k][:], wqT[128 * k : 128 * (k + 1), :])
                    for m in range(4):
                        nc.sync.dma_start(bq_sb[m][:], bqv[128 * m : 128 * (m + 1), :])
                    for m in range(4):
                        qp = qkps.tile([128, T], f32, tag="qkp")
                        for k in range(8):
                            for j in range(NJ):
                                nc.tensor.matmul(
                                    qp[:, CS[j] : CS[j + 1]],
                                    wq_sb[k][:, 128 * m : 128 * (m + 1)],
                                    xT_sb[k][:, CS[j] : CS[j + 1]],
                                    start=(k == 0), stop=(k == 7),
                                )
                        nc.scalar.activation(qT_sb[m][:], qp[:], IDENT, bias=bq_sb[m][:])

                    wk_sb = [wq.tile([128, FPC], f32r, name=f"wk{k}") for k in range(8)]
                    for k in range(8):
                        nc.sync.dma_start(wk_sb[k][:], wkT[128 * k : 128 * (k + 1), :])
                    for m in range(4):
                        kp = qkps.tile([128, T], f32, tag="qkp")
                        for k in range(8):
                            for j in range(NJ):
                                nc.tensor.matmul(
                                    kp[:, CS[j] : CS[j + 1]],
                                    wk_sb[k][:, 128 * m : 128 * (m + 1)],
                                    xT_sb[k][:, CS[j] : CS[j + 1]],
                                    start=(k == 0), stop=(k == 7),
                                )
                        nc.vector.tensor_copy(kT_sb[m][:], kp[:])

                # v = x @ Wv_p^T + bv; stored interleaved [v_h (64) | 1] * 8.
                # bv is added on evacuation (partition-broadcast once); the
                # ones columns come from a tiny constant DMA.
                with tc.tile_pool(name="wv", bufs=1) as wv, \
                     tc.tile_pool(name="vps", bufs=2, space="PSUM") as vps:
                    wv_sb = [wv.tile([128, FPC], f32r, name=f"wv{k}") for k in range(8)]
                    for k in range(8):
                        nc.sync.dma_start(wv_sb[k][:], wvT[128 * k : 128 * (k + 1), :])
                    bv_row = wv.tile([1, FPC], f32, name="bv_row")
                    nc.sync.dma_start(bv_row[:], bvv[:])
                    bv_bc = wv.tile([128, FPC], f32, name="bv_bc")
                    nc.gpsimd.partition_broadcast(bv_bc[:], bv_row[:])
                    for i in range(NT):
                        rw = _trows(i)
                        nc.sync.dma_start(
                            v_sb[i][0:rw].rearrange("p (h c) -> p h c", c=VW)[:, :, DH : DH + 1],
                            vones[0:rw],
                        )
                        vp = vps.tile([128, FPC], f32, tag="vp")
                        for k in range(8):
                            nc.tensor.matmul(
                                vp[0:rw, :],
                                xT_sb[k][:, 128 * i : 128 * i + rw],
                                wv_sb[k][:],
                                start=(k == 0), stop=(k == 7),
                            )
                        nc.vector.tensor_add(
                            v_sb[i][0:rw].rearrange("p (h c) -> p h c", c=VW)[:, :, 0:DH],
                            vp[0:rw].rearrange("p (h c) -> p h c", c=DH),
                            bv_bc[0:rw].rearrange("p (h c) -> p h c", c=DH),
                        )

            # ---------------- phase 2: attention, head pairs ----------------
            # pair t: head A = 2t (partitions 0:64), head B = 2t+1 (64:128)
            with tc.tile_pool(name="sstage", bufs=4) as sstage, \
                 tc.tile_pool(name="et", bufs=4) as etp, \
                 tc.tile_pool(name="nrm", bufs=2) as nrm, \
                 tc.tile_pool(name="wo", bufs=1) as wo, \
                 tc.tile_pool(name="ostage", bufs=2) as ostage, \
                 tc.tile_pool(name="sps", bufs=1, space="PSUM") as sps, \
                 tc.tile_pool(name="stps", bufs=2, space="PSUM") as stps, \
                 tc.tile_pool(name="pops", bufs=1, space="PSUM") as pops:
                wo_sb = [wo.tile([128, D], f32r, name=f"wo{k}") for k in range(4)]
                for k in range(4):
                    nc.sync.dma_start(wo_sb[k][:], woT[128 * k : 128 * (k + 1), :])
                for t in range(4):
                    hA = 2 * t
                    qt, kt = qT_sb[t], kT_sb[t]

                    # --- scores S[tq, tk] for the qk output (valid prefix) ---
                    # paired: bank0 = head A, bank1 = head B; st cols
                    # [0,T) = A, [T,2T) = B
                    for i in range(NT):
                        rw = _trows(i)
                        vend = min(128 * i + 128, T)
                        stA = sstage.tile([128, T], f32, tag="sstage")
                        stB = sstage.tile([128, T], f32, tag="sstage")
                        for j in range(NJ):
                            if CS[j] >= vend:
                                break
                            ce = min(CS[j + 1], vend)
                            w = ce - CS[j]
                            # separate psum tiles per head: same-tensor writes
                            # get serialized by the dep tracker, separate
                            # tiles let the two K=64 row-group matmuls run
                            # concurrently in the PE array (2x).
                            spA = sps.tile([128, 512], f32, tag="spA")
                            spB = sps.tile([128, 512], f32, tag="spB")
                            nc.tensor.matmul(
                                spA[0:rw, 0 : CS[j + 1] - CS[j]],
                                qt[0:64, 128 * i : 128 * i + rw],
                                kt[0:64, CS[j] : CS[j + 1]],
                                start=True, stop=True, tile_position=(0, 0),
                            )
                            nc.tensor.matmul(
                                spB[0:rw, 0 : CS[j + 1] - CS[j]],
                                qt[64:128, 128 * i : 128 * i + rw],
                                kt[64:128, CS[j] : CS[j + 1]],
                                start=True, stop=True, tile_position=(64, 0),
                            )
                            # ~25% of evacuation columns go to ACT, rest DVE
                            if (i * NJ + j) % 2 == 0:
                                nc.scalar.activation(stA[0:rw, CS[j] : ce], spA[0:rw, 0:w], IDENT)
                            else:
                                nc.vector.tensor_copy(stA[0:rw, CS[j] : ce], spA[0:rw, 0:w])
                            nc.vector.tensor_copy(stB[0:rw, CS[j] : ce], spB[0:rw, 0:w])
                        # causal -inf fill on the diagonal window [128i, vend)
                        for st, h in ((stA, hA), (stB, hA + 1)):
                            nc.gpsimd.affine_select(
                                st[0:rw, 128 * i : vend],
                                st[0:rw, 128 * i : vend],
                                pattern=[[-1, vend - 128 * i]],
                                compare_op=GE, fill=float("-inf"),
                                base=0, channel_multiplier=1,
                            )
                            nc.sync.dma_start(
                                qk_out[h, 128 * i : 128 * i + rw, 0:vend],
                                st[0:rw, 0:vend],
                            )

                    # --- S^T -> exp -> E^T -> po += v_aug.T @ E^T, paired ---
                    for j in range(NJ):
                        wj = CS[j + 1] - CS[j]
                        po = pops.tile([VW, 1024], f32, tag="po")
                        ms = [m for m in range(NT) if 128 * m < CS[j + 1]]
                        for n, m in enumerate(ms):
                            rw = _trows(m)
                            c0 = 128 * m
                            stp = stps.tile([128, 1024], f32, tag="stp")
                            nc.tensor.matmul(
                                stp[0:rw, 0:wj],
                                kt[0:64, c0 : c0 + rw],
                                qt[0:64, CS[j] : CS[j + 1]],
                                start=True, stop=True, tile_position=(0, 0),
                            )
                            nc.tensor.matmul(
                                stp[0:rw, 512 : 512 + wj],
                                kt[64:128, c0 : c0 + rw],
                                qt[64:128, CS[j] : CS[j + 1]],
                                start=True, stop=True, tile_position=(64, 0),
                            )
                            et = etp.tile([128, 1024], f32r, tag="et")
                            cs = max(c0 - CS[j], 0)      # in-tile valid start
                            nc.scalar.activation(
                                et[0:rw].rearrange("p (g c) -> p g c", c=512)[:, :, cs:wj],
                                stp[0:rw].rearrange("p (g c) -> p g c", c=512)[:, :, cs:wj],
                                EXP,
                            )
                            # zero strictly-below-diagonal (and the unwritten
                            # [0, cs) prefix): keep where tq - tk >= 0
                            me = min(c0 + 128, CS[j + 1]) - CS[j]
                            if c0 + 128 > CS[j] and me > 0:
                                nc.gpsimd.affine_select(
                                    et[0:rw].rearrange("p (g c) -> p g c", c=512)[:, :, 0:me],
                                    et[0:rw].rearrange("p (g c) -> p g c", c=512)[:, :, 0:me],
                                    pattern=[[0, 2], [1, me]],
                                    compare_op=GE, fill=0.0,
                                    base=CS[j] - c0, channel_multiplier=-1,
                                )
                            nc.tensor.matmul(
                                po[:, 0:wj],
                                v_sb[m][0:rw, VW * hA : VW * hA + VW],
                                et[0:rw, 0:wj],
                                start=(n == 0), stop=(n == len(ms) - 1),
                            )
                            nc.tensor.matmul(
                                po[:, 512 : 512 + wj],
                                v_sb[m][0:rw, VW * (hA + 1) : VW * (hA + 1) + VW],
                                et[0:rw, 512 : 512 + wj],
                                start=(n == 0), stop=(n == len(ms) - 1),
                            )

                        # --- normalize: out_h^T[:, j] = po[0:64] * (1/po[64]) ---
                        # d >= min exp > 0 and bounded, so the fast
                        # bit-trick reciprocal (~18 correct bits) is safe;
                        # DVE InstReciprocal is ~6.4 ns/element, way too slow.
                        dsb = nrm.tile([1, 1024], f32, tag="dsb")
                        rsb = nrm.tile([1, 1024], f32, tag="rsb")
                        pod = po[DH : DH + 1].rearrange("p (g c) -> p g c", c=512)[:, :, 0:wj]
                        dsbv = dsb.rearrange("p (g c) -> p g c", c=512)[:, :, 0:wj]
                        rsbv = rsb.rearrange("p (g c) -> p g c", c=512)[:, :, 0:wj]
                        nc.scalar.activation(dsbv, pod, IDENT)
                        nc.vector.reciprocal_approx_fast(out=rsbv, in_=dsbv)
                        rbc = nrm.tile([DH, 1024], f32, tag="rbc")
                        nc.gpsimd.partition_broadcast(rbc[:], rsb[:])
                        nc.vector.tensor_mul(
                            oT_sb[t][0:DH, CS[j] : CS[j + 1]],
                            po[0:DH, 0:wj], rbc[:, 0:wj],
                        )
                        nc.vector.tensor_mul(
                            oT_sb[t][DH:128, CS[j] : CS[j + 1]],
                            po[0:DH, 512 : 512 + wj], rbc[:, 512 : 512 + wj],
                        )

                # ---------- output projection (shares the stp psum slots,
                # so it overlaps the tail of the attention phase) ----------
                for n in range(8):
                    ppA = stps.tile([128, 1024], f32, tag="stp")  # chunks 0,1
                    ppB = stps.tile([128, 1024], f32, tag="stp")  # chunk 2
                    for k in range(4):
                        nc.tensor.matmul(
                            ppA[:, 0:512],
                            wo_sb[k][:, 128 * n : 128 * (n + 1)],
                            oT_sb[k][:, CS[0] : CS[1]],
                            start=(k == 0), stop=(k == 3),
                        )
                        nc.tensor.matmul(
                            ppA[:, 512:1024],
                            wo_sb[k][:, 128 * n : 128 * (n + 1)],
                            oT_sb[k][:, CS[1] : CS[2]],
                            start=(k == 0), stop=(k == 3),
                        )
                        nc.tensor.matmul(
                            ppB[:, 0 : CS[3] - CS[2]],
                            wo_sb[k][:, 128 * n : 128 * (n + 1)],
                            oT_sb[k][:, CS[2] : CS[3]],
                            start=(k == 0), stop=(k == 3),
                        )
                    ot = ostage.tile([128, T], f32, tag="ot")
                    if n % 2 == 0:
                        nc.scalar.activation(ot[:, 0:1024], ppA[:], IDENT)
                        nc.vector.tensor_copy(ot[:, 1024:T], ppB[:, 0 : CS[3] - CS[2]])
                    else:
                        nc.vector.tensor_copy(ot[:, 0:1024], ppA[:])
                        nc.scalar.activation(ot[:, 1024:T], ppB[:, 0 : CS[3] - CS[2]], IDENT)
                    nc.sync.dma_start(outT[128 * n : 128 * (n + 1), :], ot[:])

    nc.compile()
    return nc


def _get_nc():
    global _cached_nc
    if _cached_nc is None:
        _cached_nc = _build()
    return _cached_nc


def kernel(x, mask, Wq, bq, Wk, Wv, bv, Wo, bo, _run_kwargs=None):
    x = np.asarray(x, dtype=np.float32)
    Wq = np.asarray(Wq, dtype=np.float32)
    bq = np.asarray(bq, dtype=np.float32)
    Wk = np.asarray(Wk, dtype=np.float32)
    Wv = np.asarray(Wv, dtype=np.float32)
    bv = np.asarray(bv, dtype=np.float32)
    Wo = np.asarray(Wo, dtype=np.float32)
    bo = np.asarray(bo, dtype=np.float32)

    nc = _get_nc()
    s = float(DH) ** -0.25

    in_maps = []
    for c in range(NCORES):
        b, hg = divmod(c, 2)
        sl = slice(hg * FPC, (hg + 1) * FPC)
        in_maps.append({
            "xT": np.ascontiguousarray(x[b].T),
            "wqT": np.ascontiguousarray((Wq[sl] * s).T),
            "wkT": np.ascontiguousarray((Wk[sl] * s).T),
            "wvT": np.ascontiguousarray(Wv[sl].T),
            "woT": np.ascontiguousarray(Wo[:, sl].T),
            "bq": (bq[sl] * s).reshape(FPC, 1).astype(np.float32),
            "bv": bv[sl].reshape(1, FPC).astype(np.float32),
            "vones": np.ones((128, HPC, 1), np.float32),
        })

    res = bass_utils.run_bass_kernel_spmd(
        nc, in_maps, core_ids=list(range(NCORES)), **(_run_kwargs or {})
    )

    out = np.empty((B, T, D), np.float32)
    qk = np.empty((B, H, T, T), np.float32)
    for b in range(B):
        r0 = res.results[2 * b]
        r1 = res.results[2 * b + 1]
        out[b] = r0["outT"].T + r1["outT"].T + bo
        for hg, r in ((0, r0), (1, r1)):
            for hl in range(HPC):
                h = hg * HPC + hl
                dst = qk[b, h]
                src = r["qk_out"][hl]
                for i in range(NT):
                    ra, rb = 128 * i, 128 * i + _trows(i)
                    vend = min(128 * i + 128, T)
                    dst[ra:rb, :vend] = src[ra:rb, :vend]
                    dst[ra:rb, vend:] = -np.inf
    if _run_kwargs is not None:
        return (out, qk), res
    return out, qk
